# revision 38
# baseline (speedup 1.0000x reference)
"""Trainium2 Bass kernel for a diagonal selective SSM layer.

Reference computation (per batch element b):
    alpha = sigmoid(x @ Wg.T + bg)        # (L, S)
    u     = x @ WB.T + bB                 # (L, S)
    h_t   = alpha_t * h_{t-1} + u_t       # scan over L, h in R^S
    y     = h @ WC.T + bC                 # (L, D)

Sharding: data-parallel over batch. B == 8 == n_cores, so each NeuronCore
processes exactly one batch element; the small projection weights are
replicated to every core. No collectives needed.

Per-core dataflow:
  - G/U GEMMs in bf16 (x, Wg, WB shipped as bf16: halves HBM traffic,
    enables FWL weight loads; the PE streams 1 col/cycle for bf16 and f32r
    alike so matmul time is unchanged). PSUM fp32 accumulation.
  - All DRAM operands are shipped in SBUF layout with chunk-major (x) /
    chunk-major (y) blocking so every DMA issue is a plain 2D contiguous
    copy with 4-8KB lines (short lines drain at ~1/3 peak).
  - Input DMAs go on one HWDGE queue in exact consumption order (both
    queues share the same 16 DMA engines, so a second queue only dilutes
    the critical transfers); the final writeback is split across both
    rings so its halves drain in parallel.
  - alpha eviction: ScalarE Sigmoid activation with per-partition bias.
    u eviction: VectorE tensor_scalar_add with per-partition bias.
  - Recurrence: hardware linear-recurrence nc.vector.tensor_tensor_scan
    (state = a*state + u, fp32 internal state), chunk-chained via `initial`.
  - Output GEMM transposed (yT layout, D on partitions): the scan output
    hh (S on partitions, L free) is the moving operand, WC tiles (S parts,
    D free) the stationary one.  bias bC is then per-partition and fuses
    into the PSUM eviction (split ScalarE activation / VectorE
    tensor_scalar_add), which also casts to bf16.  The host undoes the
    transpose/blocking and upcasts.
  - HAM warm-up: a burst of N=128 dummy matmuls on a memset tile keeps the
    PE busy from right after the engine preamble until the first data
    lands, so real matmuls run at 2.4 GHz from the start.
  - Y GEMMs are skewed one chunk behind the G/U GEMMs so the PE never
    waits on the scan; first/last chunks are small to shorten the
    data-arrival head and the scan->Y->writeback tail.
"""

import numpy as np

B, L, D, S = 8, 2048, 1024, 256
P = 128
NCORES = 8
KD = D // P      # 8 k-tiles over the D contraction
MS = S // P      # 2 partition groups over S
DT = D // P      # 8 output D-tiles

# L chunks: small first chunk so the first GEMM starts as soon as possible
# after the x/Wg DMAs land; small last chunk to shorten the tail.
CHUNKS = [256, 512, 512, 512, 256]
OFFS = [sum(CHUNKS[:i]) for i in range(len(CHUNKS) + 1)]
XOFF = [KD * o for o in OFFS]   # x block offsets ([q][k][l] layout)
YOFF = [DT * o for o in OFFS]   # y block offsets ([q][t][l] layout)
assert OFFS[-1] == L

WARMUP_MMS = 52  # N=128 dummy matmuls bridging preamble-end to first-data

_NC_CACHE = {}


def _build_nc():
    import concourse.mybir as mybir
    import concourse.tile as tile
    from concourse import bacc

    f32 = mybir.dt.float32
    f32r = mybir.dt.float32r
    bf16 = mybir.dt.bfloat16
    AF = mybir.ActivationFunctionType
    OP = mybir.AluOpType

    nc = bacc.Bacc("TRN2", target_bir_lowering=False, debug=False)

    xQ = nc.dram_tensor("xQ", [P, KD * L], bf16, kind="ExternalInput")
    wg = nc.dram_tensor("wgP", [P, KD * S], bf16, kind="ExternalInput")
    wb = nc.dram_tensor("wbP", [P, KD * S], bf16, kind="ExternalInput")
    wc = nc.dram_tensor("wcP", [P, MS * D], bf16, kind="ExternalInput")
    bias = nc.dram_tensor("biasP", [P, 4 + DT], f32, kind="ExternalInput")
    y = nc.dram_tensor("yQ", [P, DT * L], bf16, kind="ExternalOutput")

    with tile.TileContext(nc) as tc:
        with (
            tc.tile_pool(name="persist", bufs=1) as pp,
            tc.tile_pool(name="psum", bufs=8, space="PSUM") as psp,
        ):
            wgta = pp.tile([P, KD * S], bf16, name="wgta", tag="wgta")
            wbta = pp.tile([P, KD * S], bf16, name="wbta", tag="wbta")
            wcta = pp.tile([P, MS * D], bf16, name="wcta", tag="wcta")
            biast = pp.tile([P, 4 + DT], f32, name="biast", tag="biast")
            xsa = pp.tile([P, KD * L], bf16, name="xsa", tag="xsa")
            ysta = pp.tile([P, DT * L], bf16, name="ysta", tag="ysta")

            # PE warm-up fodder (no DMA dependencies)
            wul = pp.tile([P, P], bf16, name="wul", tag="wul")

            # Input DMAs: one queue, issued in exact consumption order
            # (both HWDGE queues share the same 16 DMA engines, so a second
            # queue only dilutes critical transfers).
            def dma_x(q):
                sl = slice(XOFF[q], XOFF[q + 1])
                nc.sync.dma_start(xsa[:, sl], xQ[:, sl])

            nc.scalar.dma_start(biast[:], bias[:, :])  # tiny; off the main queue
            nc.sync.dma_start(wgta[:], wg[:, :])
            dma_x(0)
            nc.sync.dma_start(wbta[:], wb[:, :])
            dma_x(1)
            dma_x(2)
            nc.sync.dma_start(wcta[:], wc[:, :])
            dma_x(3)
            dma_x(4)

            alpha = [pp.tile([P, L], f32, name=f"al{m}", tag=f"al{m}") for m in range(MS)]
            uu = [pp.tile([P, L], f32, name=f"uu{m}", tag=f"uu{m}") for m in range(MS)]
            hh = [pp.tile([P, L], bf16, name=f"hh{m}", tag=f"hh{m}") for m in range(MS)]

            if WARMUP_MMS:
                nc.gpsimd.memset(wul[:], 0.0)
                wps = psp.tile([P, 512], f32, name="wps", tag="ps")
                for i in range(WARMUP_MMS):
                    nc.tensor.matmul(
                        wps[:, :P], wul[:], wul[:],
                        start=(i == 0), stop=(i == WARMUP_MMS - 1),
                    )

            groups = [
                ("g", alpha, 0), ("g", alpha, 1),
                ("b", uu, 0), ("b", uu, 1),
            ]

            def emit_gu(q):
                o0, o1 = OFFS[q], OFFS[q + 1]
                cl = o1 - o0
                qs = slice(o0, o1)
                for wt, dst, m in groups:
                    wta = wgta if wt == "g" else wbta
                    ps = psp.tile([P, 512], f32, name="ps", tag="ps")
                    for k in range(KD):
                        nc.tensor.matmul(
                            ps[:, :cl],
                            wta[:, k * S + m * P:k * S + (m + 1) * P],
                            xsa[:, XOFF[q] + k * cl:XOFF[q] + (k + 1) * cl],
                            start=(k == 0),
                            stop=(k == KD - 1),
                        )
                    if wt == "g":
                        nc.scalar.activation(
                            dst[m][:, qs], ps[:, :cl], AF.Sigmoid,
                            bias=biast[:, m:m + 1], scale=1.0,
                        )
                    else:
                        nc.vector.tensor_scalar_add(
                            dst[m][:, qs], ps[:, :cl], biast[:, 2 + m:3 + m],
                        )
                # chunk-chained hardware scan: state = alpha*state + u
                for m in range(MS):
                    init = 0.0 if q == 0 else hh[m][:, o0 - 1:o0]
                    nc.vector.tensor_tensor_scan(
                        hh[m][:, qs], alpha[m][:, qs], uu[m][:, qs],
                        init, OP.mult, OP.add,
                    )

            def emit_y(q):
                o0, o1 = OFFS[q], OFFS[q + 1]
                cl = o1 - o0
                qs = slice(o0, o1)
                last = q == len(CHUNKS) - 1
                for t in range(DT):
                    ps = psp.tile([P, 512], f32, name="psy", tag="ps")
                    for m in range(MS):
                        nc.tensor.matmul(
                            ps[:, :cl],
                            wcta[:, m * D + t * P:m * D + (t + 1) * P],
                            hh[m][:, qs],
                            start=(m == 0),
                            stop=(m == MS - 1),
                        )
                    dst = ysta[:, YOFF[q] + t * cl:YOFF[q] + (t + 1) * cl]
                    bc = biast[:, 4 + t:5 + t]
                    if last and t == DT - 1:
                        # the very last eviction gates the final writeback:
                        # split it across both engines so it lands sooner
                        hl = cl // 2
                        nc.scalar.activation(
                            dst[:, :hl], ps[:, :hl], AF.Identity, bias=bc, scale=1.0
                        )
                        nc.vector.tensor_scalar_add(dst[:, hl:], ps[:, hl:cl], bc)
                    elif t % 2 == 0:
                        nc.scalar.activation(dst, ps[:, :cl], AF.Identity, bias=bc, scale=1.0)
                    else:
                        nc.vector.tensor_scalar_add(dst, ps[:, :cl], bc)
                    if t == DT // 2 - 1:
                        # first-half writeback starts while the second half
                        # of this chunk's Y GEMMs still run
                        nc.sync.dma_start(
                            y[:, YOFF[q]:YOFF[q] + DT // 2 * cl],
                            ysta[:, YOFF[q]:YOFF[q] + DT // 2 * cl],
                        )
                # final chunk: second half goes out on the otherwise-idle
                # Scalar HWDGE ring so the two halves drain in parallel
                eng = nc.scalar if q == len(CHUNKS) - 1 else nc.sync
                eng.dma_start(
                    y[:, YOFF[q] + DT // 2 * cl:YOFF[q + 1]],
                    ysta[:, YOFF[q] + DT // 2 * cl:YOFF[q + 1]],
                )

            # software pipeline: Y GEMMs run one chunk behind G/U GEMMs so
            # the PE never waits on the scan.
            emit_gu(0)
            for q in range(1, len(CHUNKS)):
                emit_gu(q)
                emit_y(q - 1)
            emit_y(len(CHUNKS) - 1)

    nc.finalize()
    return nc


def _get_nc():
    if "nc" not in _NC_CACHE:
        _NC_CACHE["nc"] = _build_nc()
    return _NC_CACHE["nc"]


def _make_in_maps(x, Wg, bg, WB, bB, WC, bC):
    import ml_dtypes

    bf16 = ml_dtypes.bfloat16
    x = np.asarray(x, dtype=np.float32)
    # SBUF layouts: [P, KD*S] with (p, k, s) -> Wg.T[k*P+p, s], and
    # [P, MS*D] with (p, m, d) -> WC.T[m*P+p, d]
    wgP = np.ascontiguousarray(
        np.asarray(Wg, dtype=np.float32).T.astype(bf16)
        .reshape(KD, P, S).transpose(1, 0, 2).reshape(P, KD * S)
    )
    wbP = np.ascontiguousarray(
        np.asarray(WB, dtype=np.float32).T.astype(bf16)
        .reshape(KD, P, S).transpose(1, 0, 2).reshape(P, KD * S)
    )
    wcP = np.ascontiguousarray(
        np.asarray(WC, dtype=np.float32).T.astype(bf16)
        .reshape(MS, P, D).transpose(1, 0, 2).reshape(P, MS * D)
    )
    bias = np.zeros((P, 4 + DT), dtype=np.float32)
    bias[:, 0] = np.asarray(bg, dtype=np.float32)[0:P]
    bias[:, 1] = np.asarray(bg, dtype=np.float32)[P:2 * P]
    bias[:, 2] = np.asarray(bB, dtype=np.float32)[0:P]
    bias[:, 3] = np.asarray(bB, dtype=np.float32)[P:2 * P]
    bias[:, 4:] = np.asarray(bC, dtype=np.float32).reshape(DT, P).T
    in_maps = []
    for b in range(NCORES):
        # x block layout: [P, sum_q KD*cl_q], block q = (p, k, l) with
        # xQ[p, XOFF[q] + k*cl + l] = x[b][OFFS[q]+l, k*P+p]
        xk = np.ascontiguousarray(x[b].T.astype(bf16)).reshape(KD, P, L)
        blocks = [
            xk[:, :, OFFS[q]:OFFS[q + 1]].transpose(1, 0, 2).reshape(P, -1)
            for q in range(len(CHUNKS))
        ]
        xQ = np.ascontiguousarray(np.concatenate(blocks, axis=1))
        in_maps.append({
            "xQ": xQ,
            "wgP": wgP,
            "wbP": wbP,
            "wcP": wcP,
            "biasP": bias,
        })
    return in_maps


def _run(in_maps, **kwargs):
    from concourse.bass_utils import run_bass_kernel_spmd

    nc = _get_nc()
    return run_bass_kernel_spmd(nc, in_maps, list(range(NCORES)), **kwargs)


def kernel(x, Wg, bg, WB, bB, WC, bC):
    res = _run(_make_in_maps(x, Wg, bg, WB, bB, WC, bC))
    out = np.empty((NCORES, L, D), dtype=np.float32)
    for b in range(NCORES):
        yq = np.asarray(res.results[b]["yQ"])
        for q in range(len(CHUNKS)):
            o0, o1 = OFFS[q], OFFS[q + 1]
            cl = o1 - o0
            blk = yq[:, YOFF[q]:YOFF[q + 1]].reshape(P, DT, cl)
            # yQ[p, t, l] = y[o0+l, t*P+p]
            out[b, o0:o1, :] = blk.transpose(2, 1, 0).reshape(cl, D).astype(np.float32)
    return out


# revision 39
# speedup vs baseline: 1.0033x; 1.0033x over previous
"""Trainium2 Bass kernel for a diagonal selective SSM layer.

Reference computation (per batch element b):
    alpha = sigmoid(x @ Wg.T + bg)        # (L, S)
    u     = x @ WB.T + bB                 # (L, S)
    h_t   = alpha_t * h_{t-1} + u_t       # scan over L, h in R^S
    y     = h @ WC.T + bC                 # (L, D)

Sharding: data-parallel over batch. B == 8 == n_cores, so each NeuronCore
processes exactly one batch element; the small projection weights are
replicated to every core. No collectives needed.

Per-core dataflow:
  - G/U GEMMs in bf16 (x, Wg, WB shipped as bf16: halves HBM traffic,
    enables FWL weight loads; the PE streams 1 col/cycle for bf16 and f32r
    alike so matmul time is unchanged). PSUM fp32 accumulation.
  - All DRAM operands are shipped in SBUF layout with chunk-major (x) /
    chunk-major (y) blocking so every DMA issue is a plain 2D contiguous
    copy with 4-8KB lines (short lines drain at ~1/3 peak).
  - Input DMAs go on one HWDGE queue in exact consumption order (both
    queues share the same 16 DMA engines, so a second queue only dilutes
    the critical transfers); the final writeback is split across both
    rings so its halves drain in parallel.
  - alpha eviction: ScalarE Sigmoid activation with per-partition bias.
    u eviction: VectorE tensor_scalar_add with per-partition bias.
  - Recurrence: hardware linear-recurrence nc.vector.tensor_tensor_scan
    (state = a*state + u, fp32 internal state), chunk-chained via `initial`.
  - Output GEMM transposed (yT layout, D on partitions): the scan output
    hh (S on partitions, L free) is the moving operand, WC tiles (S parts,
    D free) the stationary one.  bias bC is then per-partition and fuses
    into the PSUM eviction (split ScalarE activation / VectorE
    tensor_scalar_add), which also casts to bf16.  The host undoes the
    transpose/blocking and upcasts.
  - HAM warm-up: a burst of N=128 dummy matmuls on a memset tile keeps the
    PE busy from right after the engine preamble until the first data
    lands, so real matmuls run at 2.4 GHz from the start.
  - Y GEMMs are skewed one chunk behind the G/U GEMMs so the PE never
    waits on the scan; first/last chunks are small to shorten the
    data-arrival head and the scan->Y->writeback tail.
"""

import numpy as np

B, L, D, S = 8, 2048, 1024, 256
P = 128
NCORES = 8
KD = D // P      # 8 k-tiles over the D contraction
MS = S // P      # 2 partition groups over S
DT = D // P      # 8 output D-tiles

# L chunks: small first chunk so the first GEMM starts as soon as possible
# after the x/Wg DMAs land; small last chunk to shorten the tail.
CHUNKS = [256, 512, 512, 512, 256]
OFFS = [sum(CHUNKS[:i]) for i in range(len(CHUNKS) + 1)]
XOFF = [KD * o for o in OFFS]   # x block offsets ([q][k][l] layout)
YOFF = [DT * o for o in OFFS]   # y block offsets ([q][t][l] layout)
assert OFFS[-1] == L

WARMUP_MMS = 52  # N=128 dummy matmuls bridging preamble-end to first-data

_NC_CACHE = {}


def _build_nc():
    import concourse.mybir as mybir
    import concourse.tile as tile
    from concourse import bacc

    f32 = mybir.dt.float32
    f32r = mybir.dt.float32r
    bf16 = mybir.dt.bfloat16
    AF = mybir.ActivationFunctionType
    OP = mybir.AluOpType

    nc = bacc.Bacc("TRN2", target_bir_lowering=False, debug=False)

    xQ = nc.dram_tensor("xQ", [P, KD * L], bf16, kind="ExternalInput")
    wg = nc.dram_tensor("wgP", [P, KD * S], bf16, kind="ExternalInput")
    wb = nc.dram_tensor("wbP", [P, KD * S], bf16, kind="ExternalInput")
    wc = nc.dram_tensor("wcP", [P, MS * D], bf16, kind="ExternalInput")
    bias = nc.dram_tensor("biasP", [P, 4 + DT], f32, kind="ExternalInput")
    y = nc.dram_tensor("yQ", [P, DT * L], bf16, kind="ExternalOutput")

    with tile.TileContext(nc) as tc:
        with (
            tc.tile_pool(name="persist", bufs=1) as pp,
            tc.tile_pool(name="psum", bufs=8, space="PSUM") as psp,
        ):
            wgta = pp.tile([P, KD * S], bf16, name="wgta", tag="wgta")
            wbta = pp.tile([P, KD * S], bf16, name="wbta", tag="wbta")
            wcta = pp.tile([P, MS * D], bf16, name="wcta", tag="wcta")
            biast = pp.tile([P, 4 + DT], f32, name="biast", tag="biast")
            xsa = pp.tile([P, KD * L], bf16, name="xsa", tag="xsa")
            ysta = pp.tile([P, DT * L], bf16, name="ysta", tag="ysta")

            # PE warm-up fodder (no DMA dependencies)
            wul = pp.tile([P, P], bf16, name="wul", tag="wul")

            # Input DMAs: one queue, issued in exact consumption order
            # (both HWDGE queues share the same 16 DMA engines, so a second
            # queue only dilutes critical transfers).
            def dma_x(q):
                sl = slice(XOFF[q], XOFF[q + 1])
                nc.sync.dma_start(xsa[:, sl], xQ[:, sl])

            nc.scalar.dma_start(biast[:], bias[:, :])  # tiny; off the main queue
            MH = KD * P  # one m-half of a G/U weight tile
            nc.sync.dma_start(wgta[:, :MH], wg[:, :MH])
            dma_x(0)
            nc.sync.dma_start(wbta[:, :MH], wb[:, :MH])
            nc.sync.dma_start(wgta[:, MH:], wg[:, MH:])
            nc.sync.dma_start(wbta[:, MH:], wb[:, MH:])
            dma_x(1)
            dma_x(2)
            nc.sync.dma_start(wcta[:], wc[:, :])
            dma_x(3)
            dma_x(4)

            alpha = [pp.tile([P, L], f32, name=f"al{m}", tag=f"al{m}") for m in range(MS)]
            uu = [pp.tile([P, L], f32, name=f"uu{m}", tag=f"uu{m}") for m in range(MS)]
            hh = [pp.tile([P, L], bf16, name=f"hh{m}", tag=f"hh{m}") for m in range(MS)]

            if WARMUP_MMS:
                nc.gpsimd.memset(wul[:], 0.0)
                wps = psp.tile([P, 512], f32, name="wps", tag="ps")
                for i in range(WARMUP_MMS):
                    nc.tensor.matmul(
                        wps[:, :P], wul[:], wul[:],
                        start=(i == 0), stop=(i == WARMUP_MMS - 1),
                    )

            groups = [
                ("g", alpha, 0), ("b", uu, 0),
                ("g", alpha, 1), ("b", uu, 1),
            ]

            def emit_gu(q):
                o0, o1 = OFFS[q], OFFS[q + 1]
                cl = o1 - o0
                qs = slice(o0, o1)
                for wt, dst, m in groups:
                    wta = wgta if wt == "g" else wbta
                    ps = psp.tile([P, 512], f32, name="ps", tag="ps")
                    for k in range(KD):
                        nc.tensor.matmul(
                            ps[:, :cl],
                            wta[:, m * MH + k * P:m * MH + (k + 1) * P],
                            xsa[:, XOFF[q] + k * cl:XOFF[q] + (k + 1) * cl],
                            start=(k == 0),
                            stop=(k == KD - 1),
                        )
                    if wt == "g":
                        nc.scalar.activation(
                            dst[m][:, qs], ps[:, :cl], AF.Sigmoid,
                            bias=biast[:, m:m + 1], scale=1.0,
                        )
                    else:
                        nc.vector.tensor_scalar_add(
                            dst[m][:, qs], ps[:, :cl], biast[:, 2 + m:3 + m],
                        )
                # chunk-chained hardware scan: state = alpha*state + u
                for m in range(MS):
                    init = 0.0 if q == 0 else hh[m][:, o0 - 1:o0]
                    nc.vector.tensor_tensor_scan(
                        hh[m][:, qs], alpha[m][:, qs], uu[m][:, qs],
                        init, OP.mult, OP.add,
                    )

            def emit_y(q):
                o0, o1 = OFFS[q], OFFS[q + 1]
                cl = o1 - o0
                qs = slice(o0, o1)
                last = q == len(CHUNKS) - 1
                for t in range(DT):
                    ps = psp.tile([P, 512], f32, name="psy", tag="ps")
                    for m in range(MS):
                        nc.tensor.matmul(
                            ps[:, :cl],
                            wcta[:, m * D + t * P:m * D + (t + 1) * P],
                            hh[m][:, qs],
                            start=(m == 0),
                            stop=(m == MS - 1),
                        )
                    dst = ysta[:, YOFF[q] + t * cl:YOFF[q] + (t + 1) * cl]
                    bc = biast[:, 4 + t:5 + t]
                    if last and t == DT - 1:
                        # the very last eviction gates the final writeback:
                        # split it across both engines so it lands sooner
                        hl = cl // 2
                        nc.scalar.activation(
                            dst[:, :hl], ps[:, :hl], AF.Identity, bias=bc, scale=1.0
                        )
                        nc.vector.tensor_scalar_add(dst[:, hl:], ps[:, hl:cl], bc)
                    elif t % 2 == 0:
                        nc.scalar.activation(dst, ps[:, :cl], AF.Identity, bias=bc, scale=1.0)
                    else:
                        nc.vector.tensor_scalar_add(dst, ps[:, :cl], bc)
                    if t == DT // 2 - 1:
                        # first-half writeback starts while the second half
                        # of this chunk's Y GEMMs still run
                        nc.sync.dma_start(
                            y[:, YOFF[q]:YOFF[q] + DT // 2 * cl],
                            ysta[:, YOFF[q]:YOFF[q] + DT // 2 * cl],
                        )
                # final chunk: second half goes out on the otherwise-idle
                # Scalar HWDGE ring so the two halves drain in parallel
                eng = nc.scalar if q == len(CHUNKS) - 1 else nc.sync
                eng.dma_start(
                    y[:, YOFF[q] + DT // 2 * cl:YOFF[q + 1]],
                    ysta[:, YOFF[q] + DT // 2 * cl:YOFF[q + 1]],
                )

            # software pipeline: Y GEMMs run one chunk behind G/U GEMMs so
            # the PE never waits on the scan.
            emit_gu(0)
            for q in range(1, len(CHUNKS)):
                emit_gu(q)
                emit_y(q - 1)
            emit_y(len(CHUNKS) - 1)

    nc.finalize()
    return nc


def _get_nc():
    if "nc" not in _NC_CACHE:
        _NC_CACHE["nc"] = _build_nc()
    return _NC_CACHE["nc"]


def _make_in_maps(x, Wg, bg, WB, bB, WC, bC):
    import ml_dtypes

    bf16 = ml_dtypes.bfloat16
    x = np.asarray(x, dtype=np.float32)
    # SBUF layouts: G/U weights m-major [P, MS*KD*P] with
    # (p, m, k, j) -> W.T[k*P+p, m*P+j] so each m-half is one contiguous
    # DMA; WC stays [P, MS*D] with (p, m, d) -> WC.T[m*P+p, d]
    wgP = np.ascontiguousarray(
        np.asarray(Wg, dtype=np.float32).T.astype(bf16)
        .reshape(KD, P, MS, P).transpose(1, 2, 0, 3).reshape(P, MS * KD * P)
    )
    wbP = np.ascontiguousarray(
        np.asarray(WB, dtype=np.float32).T.astype(bf16)
        .reshape(KD, P, MS, P).transpose(1, 2, 0, 3).reshape(P, MS * KD * P)
    )
    wcP = np.ascontiguousarray(
        np.asarray(WC, dtype=np.float32).T.astype(bf16)
        .reshape(MS, P, D).transpose(1, 0, 2).reshape(P, MS * D)
    )
    bias = np.zeros((P, 4 + DT), dtype=np.float32)
    bias[:, 0] = np.asarray(bg, dtype=np.float32)[0:P]
    bias[:, 1] = np.asarray(bg, dtype=np.float32)[P:2 * P]
    bias[:, 2] = np.asarray(bB, dtype=np.float32)[0:P]
    bias[:, 3] = np.asarray(bB, dtype=np.float32)[P:2 * P]
    bias[:, 4:] = np.asarray(bC, dtype=np.float32).reshape(DT, P).T
    in_maps = []
    for b in range(NCORES):
        # x block layout: [P, sum_q KD*cl_q], block q = (p, k, l) with
        # xQ[p, XOFF[q] + k*cl + l] = x[b][OFFS[q]+l, k*P+p]
        xk = np.ascontiguousarray(x[b].T.astype(bf16)).reshape(KD, P, L)
        blocks = [
            xk[:, :, OFFS[q]:OFFS[q + 1]].transpose(1, 0, 2).reshape(P, -1)
            for q in range(len(CHUNKS))
        ]
        xQ = np.ascontiguousarray(np.concatenate(blocks, axis=1))
        in_maps.append({
            "xQ": xQ,
            "wgP": wgP,
            "wbP": wbP,
            "wcP": wcP,
            "biasP": bias,
        })
    return in_maps


def _run(in_maps, **kwargs):
    from concourse.bass_utils import run_bass_kernel_spmd

    nc = _get_nc()
    return run_bass_kernel_spmd(nc, in_maps, list(range(NCORES)), **kwargs)


def kernel(x, Wg, bg, WB, bB, WC, bC):
    res = _run(_make_in_maps(x, Wg, bg, WB, bB, WC, bC))
    out = np.empty((NCORES, L, D), dtype=np.float32)
    for b in range(NCORES):
        yq = np.asarray(res.results[b]["yQ"])
        for q in range(len(CHUNKS)):
            o0, o1 = OFFS[q], OFFS[q + 1]
            cl = o1 - o0
            blk = yq[:, YOFF[q]:YOFF[q + 1]].reshape(P, DT, cl)
            # yQ[p, t, l] = y[o0+l, t*P+p]
            out[b, o0:o1, :] = blk.transpose(2, 1, 0).reshape(cl, D).astype(np.float32)
    return out
